# revision 1
# baseline (speedup 1.0000x reference)
"""Trainium2 Bass kernel for nn_BoundaryModule_38422777430159.

Reference computation (B=4, C=256, T=256, N=10, D=40, DIM0=512, DIM1=128):
  x1 = sample(feature)            # (B,C,N,D,T) via (T, N*D*T) smp matmul
  x2 = leaky(einsum('bcndt,ocn->bodt', x1, w0) + b0)
  x3 = leaky(w1 @ x2 + b1)        # 1x1 conv
  x4 = leaky(conv3x3(x3, w2) + b2)
  out = sigmoid(w3 @ x4 + b3)     # (B, D, T)

Device strategy (8 NeuronCores, SPMD; core i handles b = i//2 and
t-half th = i%2 with a 1-column halo):
  A[n]   = feature[b].T-contraction with w0[:, :, n]   (PE, fp32r)
  x2     = sum over (n, tau-chunk) of A-tiles @ W-slice (PE, fp32r)
           where W-slice is the dense (2560, 40*130) sampling matrix
           columns for this core's t-window, streamed from HBM
  x3, conv3x3, final 1x1 + sigmoid on-core; output (40, 128) per core.

fp32r (TF32-like fast fp32 path, 1 cyc/row vs 4 for fp32) is used for all
matmuls; PSUM accumulates in fp32.
"""
import os
import sys

for _p in ("/opt/trn_rl_repo", "/root/.axon_site/_ro/trn_rl_repo"):
    if os.path.isdir(_p) and _p not in sys.path:
        sys.path.append(_p)

import numpy as np

import concourse.bass as bass
import concourse.tile as tile
from concourse import mybir
from concourse.bass_utils import run_bass_kernel_spmd
from concourse.tile_rust import add_dep_helper

T = 256
N = 10
D = 40
B = 4
C_IN = 256
DIM0 = 512
DIM1 = 128

TW = 130          # t-window incl. 1-col halo each side
COLS = D * TW     # 5200 matmul columns per core
FW = 400          # free-dim chunk (<=512 psum bank, >=256 keeps fp32r fast)
NF = COLS // FW   # 13
K = 2 * N         # 20 contraction chunks of 128 (tau-chunk major within n)
DCH = 3           # conv d-rows per psum group
NDCH = (D + DCH - 1) // DCH  # 14 (13*3 + 1)

F32 = mybir.dt.float32
F32R = mybir.dt.float32r


def _legalize_waits(nc, limit=1):
    """This walrus build allows a single embedded sync wait per real
    instruction; move the excess onto standalone NoOp wait-carriers."""
    moved = 0
    for f in nc.m.functions:
        for bb in f.blocks:
            il = bb.instructions
            out = []
            changed = False
            for inst in il:
                si = inst.sync_info
                ty = type(inst).__name__
                if (si and si.on_wait and len(si.on_wait) > limit
                        and ty not in ("InstEventSemaphore", "InstNoOp")):
                    keep = si.on_wait[-limit:]
                    for w in si.on_wait[:-limit]:
                        out.append(mybir.InstNoOp(
                            name=f"waitnop-{nc.next_id()}",
                            sync_info=mybir.SyncInfo(on_wait=[w], on_update=[]),
                            bass_nofuse=True,
                            engine=inst.engine,
                        ))
                        moved += 1
                    inst.sync_info = mybir.SyncInfo(
                        on_wait=keep, on_update=si.on_update)
                    changed = True
                out.append(inst)
            if changed:
                bb.instructions = out
    return moved


def _build_program(keep=None, debug=False):
    if keep is None:
        keep = tuple(tuple(range(K)) for _ in range(NF))
    nc = bass.Bass(trn_type="TRN2")
    MAX = mybir.AluOpType.max
    MULT = mybir.AluOpType.mult

    feat_d = nc.dram_tensor("feat", [C_IN, T], F32R, kind="ExternalInput")
    w0_d = nc.dram_tensor("w0t", [N, C_IN, DIM0], F32R, kind="ExternalInput")
    wsmp_d = nc.dram_tensor("wsmp", [NF, K, 128, FW], F32R,
                            kind="ExternalInput")
    w1_d = nc.dram_tensor("w1t", [DIM0, DIM1], F32R, kind="ExternalInput")
    w2_d = nc.dram_tensor("w2t", [9, DIM1, DIM1], F32R, kind="ExternalInput")
    w3_d = nc.dram_tensor("w3t", [DIM1, 1], F32R, kind="ExternalInput")
    b0_d = nc.dram_tensor("b0", [4, 128, 1], F32, kind="ExternalInput")
    b1_d = nc.dram_tensor("b1", [128, 1], F32, kind="ExternalInput")
    b2_d = nc.dram_tensor("b2", [128, 1], F32, kind="ExternalInput")
    b3_d = nc.dram_tensor("b3", [1, 1], F32, kind="ExternalInput")
    out_d = nc.dram_tensor("out", [1, D * TW], F32, kind="ExternalOutput")
    if debug:
        dbg_a = nc.dram_tensor("dbg_a", [K, 128, DIM0], F32, kind="ExternalOutput")
        dbg_x2 = nc.dram_tensor("dbg_x2", [4, 128, FW], F32, kind="ExternalOutput")
        dbg_x3 = nc.dram_tensor("dbg_x3", [128, COLS], F32, kind="ExternalOutput")
        dbg_x4 = nc.dram_tensor("dbg_x4", [128, DCH * TW], F32, kind="ExternalOutput")

    with tile.TileContext(nc) as tc:
        with (
            tc.tile_pool(name="inp", bufs=1) as inp,
            tc.tile_pool(name="wst", bufs=24) as wst,
            tc.tile_pool(name="apool", bufs=1) as apool,
            tc.tile_pool(name="x2p", bufs=2) as x2p,
            tc.tile_pool(name="x3p", bufs=1) as x3p,
            tc.tile_pool(name="x4p", bufs=2) as x4p,
            tc.tile_pool(name="scr", bufs=2) as scr,
            tc.tile_pool(name="outp", bufs=1) as outp,
            tc.tile_pool(name="psb", bufs=1, space="PSUM") as psb,
            tc.tile_pool(name="psg", bufs=2, space="PSUM") as psg,
        ):
            # ---- input DMAs (all destinations write-once) ----
            feat = [inp.tile([128, T], F32R, tag=f"feat{c}", name=f"feat{c}")
                    for c in range(2)]
            for c in range(2):
                nc.sync.dma_start(feat[c][:], feat_d[c * 128:(c + 1) * 128, :])
            w0t = []
            w0_dmas = []
            for n in range(N):
                pair = []
                for c in range(2):
                    t_ = inp.tile([128, DIM0], F32R, tag=f"w0_{n}_{c}",
                                  name=f"w0_{n}_{c}")
                    w0_dmas.append(nc.sync.dma_start(
                        t_[:], w0_d[n, c * 128:(c + 1) * 128, :]))
                    pair.append(t_)
                w0t.append(pair)
            # prefetch the first two f-chunks of the W stream, each tile
            # ordered 1:1 behind the matching w0 load so stage A and stage B
            # both trickle-start as DMAs land
            wpre = {}
            for k in keep[0]:
                wt = wst.tile([128, FW], F32R, tag="w", name=f"wt_0_{k}")
                dma = nc.sync.dma_start(wt[:], wsmp_d[0, k])
                add_dep_helper(dma.ins, w0_dmas[k].ins,
                               reason="interleave W stream with w0")
                wpre[(0, k)] = wt
            w1t = []
            for c in range(4):
                t_ = inp.tile([128, DIM1], F32R, tag=f"w1_{c}", name=f"w1_{c}")
                nc.sync.dma_start(t_[:], w1_d[c * 128:(c + 1) * 128, :])
                w1t.append(t_)
            w2t = []
            for j in range(9):
                t_ = inp.tile([128, DIM1], F32R, tag=f"w2_{j}", name=f"w2_{j}")
                nc.sync.dma_start(t_[:], w2_d[j])
                w2t.append(t_)
            w3t = inp.tile([128, 1], F32R, tag="w3", name="w3t_sb")
            nc.sync.dma_start(w3t[:], w3_d[:])
            b0t = inp.tile([128, 4], F32, tag="b0", name="b0_sb")
            nc.sync.dma_start(b0t[:].rearrange("p (a b) -> p a b", b=1),
                              b0_d[:].transpose((1, 0, 2)))
            b1t = inp.tile([128, 1], F32, tag="b1", name="b1_sb")
            nc.sync.dma_start(b1t[:], b1_d[:])
            b2t = inp.tile([128, 1], F32, tag="b2", name="b2_sb")
            nc.sync.dma_start(b2t[:], b2_d[:])
            b3t = inp.tile([1, 1], F32, tag="b3", name="b3_sb")
            nc.sync.dma_start(b3t[:], b3_d[:])

            # ---- teach engines the input-DMA ticks (1 wait per inst) ----
            dve_scr = scr.tile([128, 4], F32, tag="dscr", name="dve_scr")
            nc.vector.tensor_copy(dve_scr[:, 0:1], b1t[:])
            nc.vector.tensor_copy(dve_scr[:, 1:2], b2t[:])
            nc.vector.tensor_copy(dve_scr[:, 2:3], b0t[:, 0:1])
            nc.scalar.mul(dve_scr[0:1, 3:4], b3t[:], 1.0)
            # one warm-up accumulation group, spread so stage A can start as
            # soon as the tiles it needs have landed
            warm = psg.tile([1, 4], F32, tag="g", name="warm_ps")

            def warm_mm(t_, first, last):
                nc.tensor.matmul(warm[:], t_[:, 0:1], t_[:, 0:4],
                                 start=first, stop=last)

            for i, t_ in enumerate(feat):
                warm_mm(t_, i == 0, False)

            # ---- stage A: A[k] = (feature chunk).T @ w0_n  -> [tau, o] ----
            atiles = []
            for n in range(N):
                warm_mm(w0t[n][0], False, False)
                warm_mm(w0t[n][1], False, False)
                for tch in range(2):
                    ps = psb.tile([128, DIM0], F32, tag=f"b{tch}",
                                  name=f"psa{n}_{tch}")
                    for c in range(2):
                        nc.tensor.matmul(
                            ps[:],
                            feat[c][:, tch * 128:(tch + 1) * 128],
                            w0t[n][c][:],
                            start=(c == 0), stop=(c == 1),
                        )
                    at = apool.tile([128, DIM0], F32R, tag=f"a{n}_{tch}",
                                    name=f"a{n}_{tch}")
                    nc.vector.tensor_copy(at[:], ps[:])
                    atiles.append(at)
                    if debug:
                        nc.sync.dma_start(dbg_a[n * 2 + tch],
                                          at[:].bitcast(F32))

            # ---- stages B (sampling contraction) + C (1x1) per f-chunk ----
            # och pairs double-buffered in PSUM so consecutive f-chunks overlap
            x3 = x3p.tile([128, COLS], F32R, tag="x3", name="x3_sb")
            for f in range(NF):
                ks = list(keep[f])
                wts = {}
                x2c = [None] * 4
                for g in range(2):
                    a0 = psb.tile([128, FW], F32, tag=f"b{2 * g}",
                                  name=f"psb{f}_{2 * g}")
                    a1 = psb.tile([128, FW], F32, tag=f"b{2 * g + 1}",
                                  name=f"psb{f}_{2 * g + 1}")
                    for k in ks:
                        if g == 0:
                            if f < 1:
                                wt = wpre[(f, k)]
                            else:
                                wt = wst.tile([128, FW], F32R, tag="w",
                                              name=f"wt_{f}_{k}")
                                nc.sync.dma_start(wt[:], wsmp_d[f, k])
                            wts[k] = wt
                        wt = wts[k]
                        for o, acc in ((2 * g, a0), (2 * g + 1, a1)):
                            nc.tensor.matmul(
                                acc[:],
                                atiles[k][:, o * 128:(o + 1) * 128],
                                wt[:],
                                start=(k == ks[0]), stop=(k == ks[-1]),
                            )
                    for o, acc in ((2 * g, a0), (2 * g + 1, a1)):
                        yt = x2p.tile([128, FW], F32R, tag=f"x2_{o}",
                                      name=f"x2_{f}_{o}")
                        nc.vector.tensor_scalar_add(yt[:], acc[:],
                                                    b0t[:, o:o + 1])
                        nc.vector.scalar_tensor_tensor(yt[:], yt[:], 0.01,
                                                       yt[:], MULT, MAX)
                        x2c[o] = yt
                        if debug and f == 0:
                            nc.sync.dma_start(dbg_x2[o], yt[:].bitcast(F32))
                if f == 0:
                    # late warm-ups: small weights have landed by now
                    for t_ in w1t:
                        warm_mm(t_, False, False)
                    for j, t_ in enumerate(w2t):
                        warm_mm(t_, False, j == 8)
                psc = psg.tile([128, FW], F32, tag="g", name=f"psc{f}")
                for o in range(4):
                    nc.tensor.matmul(psc[:], w1t[o][:], x2c[o][:],
                                     start=(o == 0), stop=(o == 3))
                x3f = x3[:, f * FW:(f + 1) * FW]
                nc.vector.tensor_scalar_add(x3f, psc[:], b1t[:])
                nc.vector.scalar_tensor_tensor(x3f, x3f, 0.01, x3f, MULT, MAX)

            # ---- stage D: 3x3 conv over (d, t') with zero padding ----
            if debug:
                nc.sync.dma_start(dbg_x3[:], x3[:].bitcast(F32))
            pad = x3p.tile([128, D + 2, TW + 2], F32R, tag="pad", name="padbuf")
            nc.vector.memset(pad[:].bitcast(F32), 0.0)
            x3g = x3[:].rearrange("p (d t) -> p d t", d=D)
            for dc in range(NDCH):
                d0 = dc * DCH
                nd = min(DCH, D - d0)
                nc.vector.tensor_copy(
                    pad[:, 1 + d0:1 + d0 + nd, 1:TW + 1], x3g[:, d0:d0 + nd, :])
            out_sb = outp.tile([1, D * TW], F32, tag="os", name="out_sb")
            x4cs = [None] * NDCH

            def stage_e(dc):
                d0 = dc * DCH
                fw = min(DCH, D - d0) * TW
                pse = psg.tile([1, DCH * TW], F32, tag="g", name=f"pse{dc}")
                nc.tensor.matmul(pse[:, 0:fw], w3t[:], x4cs[dc][:, 0:fw],
                                 start=True, stop=True)
                nc.scalar.activation(
                    out_sb[:, d0 * TW:d0 * TW + fw], pse[:, 0:fw],
                    mybir.ActivationFunctionType.Sigmoid,
                    bias=b3t[:], scale=1.0,
                )

            for dc in range(NDCH):
                d0 = dc * DCH
                nd = min(DCH, D - d0)
                fw = nd * TW
                psd = psg.tile([128, DCH * TW], F32, tag="d", name=f"psd{dc}")
                for j in range(9):
                    dy, dx = j // 3, j % 3
                    nc.tensor.matmul(
                        psd[:, 0:fw],
                        w2t[j][:],
                        pad[:, d0 + dy:d0 + dy + nd, dx:dx + TW],
                        start=(j == 0), stop=(j == 8),
                    )
                x4c = x4p.tile([128, DCH * TW], F32R, tag="x4", name=f"x4_{dc}")
                nc.vector.tensor_scalar_add(x4c[:, 0:fw], psd[:, 0:fw], b2t[:])
                nc.vector.scalar_tensor_tensor(x4c[:, 0:fw], x4c[:, 0:fw],
                                               0.01, x4c[:, 0:fw], MULT, MAX)
                x4cs[dc] = x4c
                if debug and dc == 0:
                    nc.sync.dma_start(dbg_x4[:], x4c[:].bitcast(F32))
                # software pipeline: E for the previous chunk runs after the
                # next conv group is queued, hiding the DVE eviction latency
                if dc >= 1:
                    stage_e(dc - 1)
            stage_e(NDCH - 1)
            nc.scalar.dma_start(out_d[:], out_sb[:])
    _legalize_waits(nc)
    return nc


_PROGRAM = None


def _get_program(keep):
    global _PROGRAM
    if _PROGRAM is None or _PROGRAM[0] != keep:
        _PROGRAM = (keep, _build_program(keep=keep))
    return _PROGRAM[1]


def _prep_inputs(feature, smp_weight, w0, b0, w1, b1, w2, b2, w3, b3):
    feature = np.ascontiguousarray(np.asarray(feature, dtype=np.float32))
    smp = np.asarray(smp_weight, dtype=np.float32).reshape(T, N, D, T)
    w0p = np.ascontiguousarray(
        np.asarray(w0, dtype=np.float32).transpose(2, 1, 0))     # (N, C, DIM0)
    w1p = np.ascontiguousarray(np.asarray(w1, dtype=np.float32).T)  # (512,128)
    w2p = np.ascontiguousarray(
        np.asarray(w2, dtype=np.float32).transpose(2, 3, 1, 0).reshape(
            9, DIM1, DIM1))                                       # (9, C, O)
    w3p = np.ascontiguousarray(np.asarray(w3, dtype=np.float32).T)  # (128,1)
    b0p = np.ascontiguousarray(
        np.asarray(b0, dtype=np.float32).reshape(4, 128, 1))
    b1p = np.asarray(b1, dtype=np.float32).reshape(128, 1)
    b2p = np.asarray(b2, dtype=np.float32).reshape(128, 1)
    b3p = np.asarray(b3, dtype=np.float32).reshape(1, 1)

    # W slices per t-half: columns t' in [t0-1, t0+129), zero-padded outside
    # [0, T). Row-major layout (n, tau) -> K=20 chunks of 128.
    wslices = []
    for th in range(2):
        t0 = th * 128
        lo, hi = t0 - 1, t0 + TW - 1
        clo, chi = max(lo, 0), min(hi, T)
        sl = np.zeros((T, N, D, TW), dtype=np.float32)
        sl[:, :, :, clo - lo:clo - lo + (chi - clo)] = smp[:, :, :, clo:chi]
        sl = sl.transpose(1, 0, 2, 3).reshape(K, 128, D * TW)
        # (NF, K, 128, FW): each streamed [128, FW] tile contiguous in DRAM
        sl = sl.reshape(K, 128, NF, FW).transpose(2, 0, 1, 3)
        wslices.append(np.ascontiguousarray(sl))
    # skip all-zero W tiles; the keep pattern is the union over both t-halves
    # so the single SPMD program stays valid for every core
    nz = (np.abs(wslices[0]).max(axis=(2, 3)) > 0) | \
         (np.abs(wslices[1]).max(axis=(2, 3)) > 0)   # (NF, K)
    keep = tuple(
        tuple(np.nonzero(nz[f])[0].tolist()) or (0,) for f in range(NF))
    return feature, w0p, w1p, w2p, w3p, b0p, b1p, b2p, b3p, wslices, keep


def kernel(feature, smp_weight, w0, b0, w1, b1, w2, b2, w3, b3,
           _trace=False):
    (feature, w0p, w1p, w2p, w3p, b0p, b1p, b2p, b3p, wslices,
     keep) = _prep_inputs(
        feature, smp_weight, w0, b0, w1, b1, w2, b2, w3, b3)

    nc = _get_program(keep)
    in_maps = []
    for core in range(8):
        b, th = core // 2, core % 2
        in_maps.append({
            "feat": np.ascontiguousarray(feature[b]),
            "w0t": w0p,
            "wsmp": wslices[th],
            "w1t": w1p,
            "w2t": w2p,
            "w3t": w3p,
            "b0": b0p,
            "b1": b1p,
            "b2": b2p,
            "b3": b3p,
        })
    res = run_bass_kernel_spmd(nc, in_maps, core_ids=list(range(8)),
                               trace=_trace)
    out = np.empty((B, D, T), dtype=np.float32)
    for core in range(8):
        b, th = core // 2, core % 2
        full = res.results[core]["out"].reshape(D, TW)
        out[b, :, th * 128:(th + 1) * 128] = full[:, 1:TW - 1]
    if _trace:
        return out, res
    return out



# revision 2
# speedup vs baseline: 1.3134x; 1.3134x over previous
"""Trainium2 Bass kernel for nn_BoundaryModule_38422777430159.

Reference computation (B=4, C=256, T=256, N=10, D=40, DIM0=512, DIM1=128):
  x1 = sample(feature)            # (B,C,N,D,T) via (T, N*D*T) smp matmul
  x2 = leaky(einsum('bcndt,ocn->bodt', x1, w0) + b0)
  x3 = leaky(w1 @ x2 + b1)        # 1x1 conv
  x4 = leaky(conv3x3(x3, w2) + b2)
  out = sigmoid(w3 @ x4 + b3)     # (B, D, T)

Device strategy (8 NeuronCores, SPMD; core i handles b = i//2 and
t-half th = i%2 with a 1-column halo):
  A[n]   = feature[b].T-contraction with w0[:, :, n]   (PE, fp32r),
           stored as one fp8-e4m3 tile [128, 20, 512] (pre-scaled by
           1/sa folded into w0 host-side)
  x2     = sampling contraction vs the dense (2560, 40*130) W slice,
           run as fp8 DoubleRow matmuls: each slot contracts a PAIR of
           128-row chunks (2 rows/cycle), W streamed from HBM in fp8
  x3, conv3x3, final 1x1 + sigmoid on-core in fp32r; the global fp8
  scale sa*sw is folded into b0/b1/b2 (divide) and w3 (multiply), so
  no extra on-chip scaling ops are needed (leaky commutes with scale).
"""
import os
import sys

for _p in ("/opt/trn_rl_repo", "/root/.axon_site/_ro/trn_rl_repo"):
    if os.path.isdir(_p) and _p not in sys.path:
        sys.path.append(_p)

import numpy as np
import ml_dtypes

import concourse.bass as bass
import concourse.tile as tile
from concourse import mybir
from concourse.bass_utils import run_bass_kernel_spmd
from concourse.tile_rust import add_dep_helper

T = 256
N = 10
D = 40
B = 4
C_IN = 256
DIM0 = 512
DIM1 = 128

TW = 130          # t-window incl. 1-col halo each side
COLS = D * TW     # 5200 matmul columns per core
FW = 400          # free-dim chunk (<=512 psum bank)
NF = COLS // FW   # 13
K = 2 * N         # 20 contraction chunks of 128 (tau-chunk major within n)
DCH = 3           # conv d-rows per psum group
NDCH = (D + DCH - 1) // DCH  # 14 (13*3 + 1)

F32 = mybir.dt.float32
F32R = mybir.dt.float32r
F8 = mybir.dt.float8e4
DR = mybir.MatmulPerfMode.DoubleRow
E4NP = ml_dtypes.float8_e4m3
Q8 = 240.0        # e4m3 max finite


def _legalize_waits(nc, limit=1):
    """This walrus build allows a single embedded sync wait per real
    instruction; move the excess onto standalone NoOp wait-carriers."""
    moved = 0
    for f in nc.m.functions:
        for bb in f.blocks:
            il = bb.instructions
            out = []
            changed = False
            for inst in il:
                si = inst.sync_info
                ty = type(inst).__name__
                if (si and si.on_wait and len(si.on_wait) > limit
                        and ty not in ("InstEventSemaphore", "InstNoOp")):
                    keep = si.on_wait[-limit:]
                    for w in si.on_wait[:-limit]:
                        out.append(mybir.InstNoOp(
                            name=f"waitnop-{nc.next_id()}",
                            sync_info=mybir.SyncInfo(on_wait=[w], on_update=[]),
                            bass_nofuse=True,
                            engine=inst.engine,
                        ))
                        moved += 1
                    inst.sync_info = mybir.SyncInfo(
                        on_wait=keep, on_update=si.on_update)
                    changed = True
                out.append(inst)
            if changed:
                bb.instructions = out
    return moved


def _build_program(pairs):
    """pairs: tuple per f-chunk of (ka, kb) index pairs; slot (f, j)
    contracts A chunks ka and kb (256 rows) against a host-packed fp8
    W pair tile via a DoubleRow matmul."""
    nslots = [len(p) for p in pairs]
    stot = sum(nslots)
    nc = bass.Bass(trn_type="TRN2")
    MAX = mybir.AluOpType.max
    MULT = mybir.AluOpType.mult

    feat_d = nc.dram_tensor("feat", [C_IN, T], F32R, kind="ExternalInput")
    w0_d = nc.dram_tensor("w0t", [N, C_IN, DIM0], F32R, kind="ExternalInput")
    wsmp_d = nc.dram_tensor("wsmp", [stot, 128, 2, FW], F8,
                            kind="ExternalInput")
    w1_d = nc.dram_tensor("w1t", [DIM0, DIM1], F32R, kind="ExternalInput")
    w2_d = nc.dram_tensor("w2t", [9, DIM1, DIM1], F32R, kind="ExternalInput")
    w3_d = nc.dram_tensor("w3t", [DIM1, 1], F32R, kind="ExternalInput")
    b0_d = nc.dram_tensor("b0", [4, 128, 1], F32, kind="ExternalInput")
    b1_d = nc.dram_tensor("b1", [128, 1], F32, kind="ExternalInput")
    b2_d = nc.dram_tensor("b2", [128, 1], F32, kind="ExternalInput")
    b3_d = nc.dram_tensor("b3", [1, 1], F32, kind="ExternalInput")
    out_d = nc.dram_tensor("out", [1, D * TW], F32, kind="ExternalOutput")

    with tile.TileContext(nc) as tc:
        with (
            tc.tile_pool(name="inp", bufs=1) as inp,
            tc.tile_pool(name="wst", bufs=24) as wst,
            tc.tile_pool(name="apool", bufs=1) as apool,
            tc.tile_pool(name="x2p", bufs=2) as x2p,
            tc.tile_pool(name="x3p", bufs=1) as x3p,
            tc.tile_pool(name="x4p", bufs=2) as x4p,
            tc.tile_pool(name="scr", bufs=2) as scr,
            tc.tile_pool(name="outp", bufs=1) as outp,
            tc.tile_pool(name="psb", bufs=1, space="PSUM") as psb,
            tc.tile_pool(name="psg", bufs=2, space="PSUM") as psg,
        ):
            # ---- input DMAs (all destinations write-once) ----
            feat = [inp.tile([128, T], F32R, tag=f"feat{c}", name=f"feat{c}")
                    for c in range(2)]
            for c in range(2):
                nc.sync.dma_start(feat[c][:], feat_d[c * 128:(c + 1) * 128, :])
            w0t = []
            w0_dmas = []
            for n in range(N):
                pair = []
                for c in range(2):
                    t_ = inp.tile([128, DIM0], F32R, tag=f"w0_{n}_{c}",
                                  name=f"w0_{n}_{c}")
                    w0_dmas.append(nc.sync.dma_start(
                        t_[:], w0_d[n, c * 128:(c + 1) * 128, :]))
                    pair.append(t_)
                w0t.append(pair)
            # prefetch the first f-chunk's W pair tiles, each ordered 1:1
            # behind the matching w0 load so stages A and B trickle-start
            wpre = {}
            for j in range(nslots[0]):
                wt = wst.tile([128, 2, FW], F8, tag="w", name=f"wt_0_{j}")
                dma = nc.sync.dma_start(wt[:], wsmp_d[j])
                add_dep_helper(dma.ins, w0_dmas[min(j, len(w0_dmas) - 1)].ins,
                               reason="interleave W stream with w0")
                wpre[j] = wt
            w1t = []
            for c in range(4):
                t_ = inp.tile([128, DIM1], F32R, tag=f"w1_{c}", name=f"w1_{c}")
                nc.sync.dma_start(t_[:], w1_d[c * 128:(c + 1) * 128, :])
                w1t.append(t_)
            w2t = []
            for j in range(9):
                t_ = inp.tile([128, DIM1], F32R, tag=f"w2_{j}", name=f"w2_{j}")
                nc.sync.dma_start(t_[:], w2_d[j])
                w2t.append(t_)
            w3t = inp.tile([128, 1], F32R, tag="w3", name="w3t_sb")
            nc.sync.dma_start(w3t[:], w3_d[:])
            b0t = inp.tile([128, 4], F32, tag="b0", name="b0_sb")
            nc.sync.dma_start(b0t[:].rearrange("p (a b) -> p a b", b=1),
                              b0_d[:].transpose((1, 0, 2)))
            b1t = inp.tile([128, 1], F32, tag="b1", name="b1_sb")
            nc.sync.dma_start(b1t[:], b1_d[:])
            b2t = inp.tile([128, 1], F32, tag="b2", name="b2_sb")
            nc.sync.dma_start(b2t[:], b2_d[:])
            b3t = inp.tile([1, 1], F32, tag="b3", name="b3_sb")
            nc.sync.dma_start(b3t[:], b3_d[:])

            # ---- teach engines the input-DMA ticks (1 wait per inst) ----
            dve_scr = scr.tile([128, 4], F32, tag="dscr", name="dve_scr")
            nc.vector.tensor_copy(dve_scr[:, 0:1], b1t[:])
            nc.vector.tensor_copy(dve_scr[:, 1:2], b2t[:])
            nc.vector.tensor_copy(dve_scr[:, 2:3], b0t[:, 0:1])
            nc.scalar.mul(dve_scr[0:1, 3:4], b3t[:], 1.0)
            # one warm-up accumulation group, spread so stage A can start as
            # soon as the tiles it needs have landed
            warm = psg.tile([1, 4], F32, tag="g", name="warm_ps")

            def warm_mm(t_, first, last):
                nc.tensor.matmul(warm[:], t_[:, 0:1], t_[:, 0:4],
                                 start=first, stop=last)

            for i, t_ in enumerate(feat):
                warm_mm(t_, i == 0, False)

            # ---- stage A: A[k] = (feature chunk).T @ w0_n -> fp8 [tau,o] ----
            a8 = apool.tile([128, K, DIM0], F8, tag="a8", name="a8_sb")
            for n in range(N):
                warm_mm(w0t[n][0], False, False)
                warm_mm(w0t[n][1], False, False)
                for tch in range(2):
                    ps = psb.tile([128, DIM0], F32, tag=f"b{tch}",
                                  name=f"psa{n}_{tch}")
                    for c in range(2):
                        nc.tensor.matmul(
                            ps[:],
                            feat[c][:, tch * 128:(tch + 1) * 128],
                            w0t[n][c][:],
                            start=(c == 0), stop=(c == 1),
                        )
                    k = n * 2 + tch
                    nc.vector.tensor_copy(a8[:, k:k + 1, :], ps[:])

            # ---- stages B (fp8 DoubleRow sampling) + C (1x1) per f-chunk --
            # och pairs double-buffered in PSUM so consecutive f-chunks overlap
            x3 = x3p.tile([128, COLS], F32R, tag="x3", name="x3_sb")
            sbase = 0
            for f in range(NF):
                pf = pairs[f]
                wts = {}
                x2c = [None] * 4
                for g in range(2):
                    a0 = psb.tile([128, FW], F32, tag=f"b{2 * g}",
                                  name=f"psb{f}_{2 * g}")
                    a1 = psb.tile([128, FW], F32, tag=f"b{2 * g + 1}",
                                  name=f"psb{f}_{2 * g + 1}")
                    for j, (ka, kb) in enumerate(pf):
                        if g == 0:
                            if f < 1:
                                wt = wpre[j]
                            else:
                                wt = wst.tile([128, 2, FW], F8, tag="w",
                                              name=f"wt_{f}_{j}")
                                nc.sync.dma_start(wt[:], wsmp_d[sbase + j])
                            wts[j] = wt
                        wt = wts[j]
                        for o, acc in ((2 * g, a0), (2 * g + 1, a1)):
                            nc.tensor.matmul(
                                acc[:],
                                a8[:, ka:kb + 1:(kb - ka),
                                   o * 128:(o + 1) * 128],
                                wt[:],
                                start=(j == 0), stop=(j == len(pf) - 1),
                                perf_mode=DR,
                            )
                    for o, acc in ((2 * g, a0), (2 * g + 1, a1)):
                        yt = x2p.tile([128, FW], F32R, tag=f"x2_{o}",
                                      name=f"x2_{f}_{o}")
                        nc.vector.tensor_scalar_add(yt[:], acc[:],
                                                    b0t[:, o:o + 1])
                        nc.vector.scalar_tensor_tensor(yt[:], yt[:], 0.01,
                                                       yt[:], MULT, MAX)
                        x2c[o] = yt
                sbase += len(pf)
                if f == 0:
                    # late warm-ups: small weights have landed by now
                    for t_ in w1t:
                        warm_mm(t_, False, False)
                    for j, t_ in enumerate(w2t):
                        warm_mm(t_, False, j == 8)
                psc = psg.tile([128, FW], F32, tag="g", name=f"psc{f}")
                for o in range(4):
                    nc.tensor.matmul(psc[:], w1t[o][:], x2c[o][:],
                                     start=(o == 0), stop=(o == 3))
                x3f = x3[:, f * FW:(f + 1) * FW]
                nc.vector.tensor_scalar_add(x3f, psc[:], b1t[:])
                nc.vector.scalar_tensor_tensor(x3f, x3f, 0.01, x3f, MULT, MAX)

            # ---- stage D: 3x3 conv over (d, t') with zero padding ----
            pad = x3p.tile([128, D + 2, TW + 2], F32R, tag="pad", name="padbuf")
            nc.vector.memset(pad[:].bitcast(F32), 0.0)
            x3g = x3[:].rearrange("p (d t) -> p d t", d=D)
            for dc in range(NDCH):
                d0 = dc * DCH
                nd = min(DCH, D - d0)
                nc.vector.tensor_copy(
                    pad[:, 1 + d0:1 + d0 + nd, 1:TW + 1], x3g[:, d0:d0 + nd, :])
            out_sb = outp.tile([1, D * TW], F32, tag="os", name="out_sb")
            x4cs = [None] * NDCH

            def stage_e(dc):
                d0 = dc * DCH
                fw = min(DCH, D - d0) * TW
                pse = psg.tile([1, DCH * TW], F32, tag="g", name=f"pse{dc}")
                nc.tensor.matmul(pse[:, 0:fw], w3t[:], x4cs[dc][:, 0:fw],
                                 start=True, stop=True)
                nc.scalar.activation(
                    out_sb[:, d0 * TW:d0 * TW + fw], pse[:, 0:fw],
                    mybir.ActivationFunctionType.Sigmoid,
                    bias=b3t[:], scale=1.0,
                )

            for dc in range(NDCH):
                d0 = dc * DCH
                nd = min(DCH, D - d0)
                fw = nd * TW
                psd = psg.tile([128, DCH * TW], F32, tag="d", name=f"psd{dc}")
                for j in range(9):
                    dy, dx = j // 3, j % 3
                    nc.tensor.matmul(
                        psd[:, 0:fw],
                        w2t[j][:],
                        pad[:, d0 + dy:d0 + dy + nd, dx:dx + TW],
                        start=(j == 0), stop=(j == 8),
                    )
                x4c = x4p.tile([128, DCH * TW], F32R, tag="x4", name=f"x4_{dc}")
                nc.vector.tensor_scalar_add(x4c[:, 0:fw], psd[:, 0:fw], b2t[:])
                nc.vector.scalar_tensor_tensor(x4c[:, 0:fw], x4c[:, 0:fw],
                                               0.01, x4c[:, 0:fw], MULT, MAX)
                x4cs[dc] = x4c
                # software pipeline: E for the previous chunk runs after the
                # next conv group is queued, hiding the DVE eviction latency
                if dc >= 1:
                    stage_e(dc - 1)
            stage_e(NDCH - 1)
            nc.scalar.dma_start(out_d[:], out_sb[:])
    _legalize_waits(nc)
    return nc


_PROGRAM = None


def _get_program(pairs):
    global _PROGRAM
    if _PROGRAM is None or _PROGRAM[0] != pairs:
        _PROGRAM = (pairs, _build_program(pairs))
    return _PROGRAM[1]


def _prep_inputs(feature, smp_weight, w0, b0, w1, b1, w2, b2, w3, b3):
    feature = np.ascontiguousarray(np.asarray(feature, dtype=np.float32))
    smp = np.asarray(smp_weight, dtype=np.float32).reshape(T, N, D, T)
    w0 = np.asarray(w0, dtype=np.float32)
    w1p = np.ascontiguousarray(np.asarray(w1, dtype=np.float32).T)  # (512,128)
    w2p = np.ascontiguousarray(
        np.asarray(w2, dtype=np.float32).transpose(2, 3, 1, 0).reshape(
            9, DIM1, DIM1))                                       # (9, C, O)
    w3 = np.asarray(w3, dtype=np.float32)
    b0 = np.asarray(b0, dtype=np.float32)
    b1 = np.asarray(b1, dtype=np.float32)
    b2 = np.asarray(b2, dtype=np.float32)
    b3p = np.asarray(b3, dtype=np.float32).reshape(1, 1)

    # W slices per t-half: columns t' in [t0-1, t0+129), zero-padded outside
    # [0, T). Row-major layout (n, tau).
    wrows = []
    for th in range(2):
        t0 = th * 128
        lo, hi = t0 - 1, t0 + TW - 1
        clo, chi = max(lo, 0), min(hi, T)
        sl = np.zeros((T, N, D, TW), dtype=np.float32)
        sl[:, :, :, clo - lo:clo - lo + (chi - clo)] = smp[:, :, :, clo:chi]
        wrows.append(sl.transpose(1, 0, 2, 3).reshape(N * T, COLS))
    sw = max(np.abs(wr).max() for wr in wrows) / Q8

    # keep pattern: union over both halves (single SPMD program), paired
    # for DoubleRow (each slot contracts two 128-row chunks)
    nz0 = wrows[0].reshape(K, 128, NF, FW)
    nz1 = wrows[1].reshape(K, 128, NF, FW)
    pairs = []
    for f in range(NF):
        ks = sorted(set(np.nonzero(
            (np.abs(nz0[:, :, f, :]).max(axis=(1, 2)) > 0) |
            (np.abs(nz1[:, :, f, :]).max(axis=(1, 2)) > 0))[0].tolist()))
        if not ks:
            ks = [0]
        pf = []
        for i in range(0, len(ks) - 1, 2):
            pf.append((ks[i], ks[i + 1]))
        if len(ks) % 2:
            k = ks[-1]
            dummy = k - 1 if k > 0 else k + 1
            pf.append((min(k, dummy), max(k, dummy)))
        pairs.append(tuple(pf))
    pairs = tuple(pairs)

    # fp8 W pair tiles [slot, 128, 2, FW]; an odd slot's dummy half is zero
    wq = [np.asarray(wr / sw, dtype=E4NP) for wr in wrows]
    stot = sum(len(p) for p in pairs)
    wpk = [np.zeros((stot, 128, 2, FW), dtype=E4NP) for _ in range(2)]
    for th in range(2):
        nzs = (np.abs(wrows[th].reshape(K, 128, NF, FW)).max(axis=(1, 3)) > 0)
        s = 0
        for f in range(NF):
            for (ka, kb) in pairs[f]:
                for h, k in ((0, ka), (1, kb)):
                    if nzs[k, f]:
                        wpk[th][s, :, h, :] = \
                            wq[th][k * 128:(k + 1) * 128,
                                   f * FW:(f + 1) * FW]
                s += 1

    # per-batch A scale folded into w0; sa*sw folded into biases and w3
    A = np.einsum('bct,ocn->bnto', feature, w0, optimize=True)
    sas = [np.abs(A[b]).max() / Q8 for b in range(B)]
    per_b = []
    for b in range(B):
        s = sas[b] * sw
        w0p = np.ascontiguousarray(
            (w0 / sas[b]).transpose(2, 1, 0))                # (N, C, DIM0)
        per_b.append(dict(
            w0t=w0p,
            w3t=np.ascontiguousarray(w3.T * s),              # (128, 1)
            b0=np.ascontiguousarray((b0 / s).reshape(4, 128, 1)),
            b1=(b1 / s).reshape(128, 1),
            b2=(b2 / s).reshape(128, 1),
        ))
    return feature, w1p, w2p, b3p, wpk, pairs, per_b


def kernel(feature, smp_weight, w0, b0, w1, b1, w2, b2, w3, b3,
           _trace=False):
    (feature, w1p, w2p, b3p, wpk, pairs, per_b) = _prep_inputs(
        feature, smp_weight, w0, b0, w1, b1, w2, b2, w3, b3)

    nc = _get_program(pairs)
    in_maps = []
    for core in range(8):
        b, th = core // 2, core % 2
        in_maps.append({
            "feat": np.ascontiguousarray(feature[b]),
            "w0t": per_b[b]["w0t"],
            "wsmp": wpk[th],
            "w1t": w1p,
            "w2t": w2p,
            "w3t": per_b[b]["w3t"],
            "b0": per_b[b]["b0"],
            "b1": per_b[b]["b1"],
            "b2": per_b[b]["b2"],
            "b3": b3p,
        })
    res = run_bass_kernel_spmd(nc, in_maps, core_ids=list(range(8)),
                               trace=_trace)
    out = np.empty((B, D, T), dtype=np.float32)
    for core in range(8):
        b, th = core // 2, core % 2
        full = res.results[core]["out"].reshape(D, TW)
        out[b, :, th * 128:(th + 1) * 128] = full[:, 1:TW - 1]
    if _trace:
        return out, res
    return out
